# revision 2
# baseline (speedup 1.0000x reference)
"""AttentionTCCNet Trainium2 Bass kernel.

Key algebraic fact exploited: the per-step attention adds a *scalar*
(att_h) to every softmax logit, so the softmax weights -- and hence the
attended frame x_t -- are constant across the 16 recurrence steps.  The
computation therefore reduces to a ConvLSTM recurrence whose per-step cost
is a 128->512ch 5x5 conv over the hidden state (13.4 GFLOP/step), plus a
one-time x-path conv and a small CNN tail.

Device kernel: the 16-step ConvLSTM recurrence (conv as 4og x 25offset
stationary-weight matmuls in bf16, fp32 PSUM accumulation, pointwise LSTM
math on Scalar/Vector engines), producing mean-over-time hidden state.
Host: input attention prep (elementwise/stats), the tiny x-path conv, and
the CNN tail (maxpool + 2 convs + normalize), all exact fp32.

SPMD over 8 NeuronCores (replicated recurrence; output read from core 0).
"""

import numpy as np
import ml_dtypes

import concourse.bass as bass
import concourse.mybir as mybir
import concourse.tile as tile
from concourse.bass_utils import run_bass_kernel_spmd

# ---------------------------------------------------------------------------
# Workaround for this container's walrus accepting only ONE SyncWait per
# instruction: split any multi-wait instruction emitted by Tile's semaphore
# assigner into single-wait NoOp carriers inserted immediately before it.
# ---------------------------------------------------------------------------
from concourse.tile import ScopedClock

_MAX_WAITS = 1
_wsplit_counter = [0]


def _split_waits_in_list(insts):
    new = []
    for inst in insts:
        si = getattr(inst, "sync_info", None)
        if si is not None and si.on_wait and len(si.on_wait) > _MAX_WAITS:
            waits = list(si.on_wait)
            for w in waits[:-_MAX_WAITS]:
                _wsplit_counter[0] += 1
                new.append(
                    mybir.InstNoOp(
                        name=f"I-wsplit-{_wsplit_counter[0]}",
                        engine=inst.engine,
                        sync_info=mybir.SyncInfo(on_wait=[w], on_update=[]),
                    )
                )
            si.on_wait = waits[-_MAX_WAITS:]
        new.append(inst)
    insts[:] = new


_orig_lower = tile.TileContext._lower_ordered_insts


def _patched_lower(self, ordered):
    for insts in ordered.values():
        _split_waits_in_list(insts)
    return _orig_lower(self, ordered)


def _patched_drain_and_barrier(self, tick_clock, wait_clock):
    nc = self.nc
    drain_inst = nc.sync.drain()
    wait_clock.add_sem_waits(
        drain_inst.ins, ScopedClock({None: tick_clock.global_clock})
    )
    si = drain_inst.ins.sync_info
    if si is not None and si.on_wait and len(si.on_wait) > _MAX_WAITS:
        waits = list(si.on_wait)
        si.on_wait = waits[:_MAX_WAITS]
        for w in waits[_MAX_WAITS:]:
            extra = nc.sync.drain()
            extra.ins.sync_info = mybir.SyncInfo(on_wait=[w], on_update=[])
    nc.all_engine_barrier()
    assert self.sems is not None
    popped = nc._tile_sem_poison_stack.pop()
    assert popped is self._sem_poison
    nc.clear_and_free_semaphores(list(self.sems.allocated().values()))
    nc.all_engine_barrier()


if tile.TileContext._lower_ordered_insts is not _patched_lower:
    tile.TileContext._lower_ordered_insts = _patched_lower
    tile.TileContext._drain_and_barrier = _patched_drain_and_barrier

# ---------------------------------------------------------------------------

N_CORES = 8
T, HS, H, W = 16, 128, 64, 64
HW = H * W  # 4096
N_CHUNK = 8          # spatial chunks of 8 rows x 64 cols = 512 free
CH_FREE = 512
PADW = 68            # 64 + 2*2 padded layout

FP32 = mybir.dt.float32
BF16 = mybir.dt.bfloat16

_nc_cache = [None]


def build_nc():
    if _nc_cache[0] is not None:
        return _nc_cache[0]
    nc = bass.Bass(num_devices=N_CORES)
    wh_d = nc.dram_tensor("wh", [128, 4 * 25 * 128], BF16, kind="ExternalInput")
    gx_d = nc.dram_tensor("gx", [4, 128, HW], FP32, kind="ExternalInput")
    out_d = nc.dram_tensor("hmean", [128, HW], FP32, kind="ExternalOutput")

    with tile.TileContext(nc) as tc:
        with (
            tc.tile_pool(name="const", bufs=1) as cpool,
            tc.tile_pool(name="hbuf", bufs=2) as hpool,
            tc.tile_pool(name="tmp", bufs=2) as tpool,
            tc.tile_pool(name="psum", bufs=2, space="PSUM") as ppool,
        ):
            wh = cpool.tile([128, 4 * 25 * 128], BF16)
            gx = cpool.tile([128, 4, HW], FP32)
            c_st = cpool.tile([128, HW], FP32)
            hsum = cpool.tile([128, HW], FP32)
            nc.sync.dma_start(wh[:], wh_d[:])
            nc.sync.dma_start(gx[:], gx_d.ap().rearrange("a p h -> p a h"))

            h_pad = None
            for t in range(T):
                if t < T - 1:
                    h_new = hpool.tile([128, PADW, PADW], BF16, tag="hpad")
                    nc.gpsimd.memset(h_new[:], 0.0)
                else:
                    h_new = None

                for ch in range(N_CHUNK):
                    r0 = ch * 8
                    cs = ch * CH_FREE
                    acts = []  # sigmoid(i), sigmoid(f), sigmoid(o), tanh(g)
                    if t == 0:
                        # h == 0: gates are exactly gx
                        for og in range(4):
                            fn = (
                                mybir.ActivationFunctionType.Tanh
                                if og == 3
                                else mybir.ActivationFunctionType.Sigmoid
                            )
                            a = tpool.tile([128, CH_FREE], FP32, tag=f"act{og}")
                            nc.scalar.activation(
                                a[:], gx[:, og, cs : cs + CH_FREE], fn
                            )
                            acts.append(a)
                    else:
                        for og in range(4):
                            ps = ppool.tile([128, CH_FREE], FP32, tag=f"ps{og}")
                            for off in range(25):
                                ky, kx = off // 5, off % 5
                                base = (og * 25 + off) * 128
                                nc.tensor.matmul(
                                    ps[:],
                                    wh[:, base : base + 128],
                                    h_pad[:, r0 + ky : r0 + ky + 8, kx : kx + 64],
                                    start=(off == 0),
                                    stop=(off == 24),
                                )
                            g_sb = tpool.tile([128, CH_FREE], FP32, tag=f"gs{og}")
                            nc.vector.tensor_add(
                                g_sb[:], ps[:], gx[:, og, cs : cs + CH_FREE]
                            )
                            fn = (
                                mybir.ActivationFunctionType.Tanh
                                if og == 3
                                else mybir.ActivationFunctionType.Sigmoid
                            )
                            a = tpool.tile([128, CH_FREE], FP32, tag=f"act{og}")
                            nc.scalar.activation(a[:], g_sb[:], fn)
                            acts.append(a)

                    i_s, f_s, o_s, g_t = acts
                    c_sl = c_st[:, cs : cs + CH_FREE]
                    m2 = tpool.tile([128, CH_FREE], FP32, tag="m2")
                    nc.vector.tensor_mul(m2[:], i_s[:], g_t[:])
                    if t == 0:
                        nc.vector.tensor_copy(c_sl, m2[:])
                    else:
                        m1 = tpool.tile([128, CH_FREE], FP32, tag="m1")
                        nc.vector.tensor_mul(m1[:], f_s[:], c_sl)
                        nc.vector.tensor_add(c_sl, m1[:], m2[:])
                    tc_t = tpool.tile([128, CH_FREE], FP32, tag="tc")
                    nc.scalar.activation(
                        tc_t[:], c_sl, mybir.ActivationFunctionType.Tanh
                    )
                    hf = tpool.tile([128, CH_FREE], FP32, tag="hf")
                    nc.vector.tensor_mul(hf[:], o_s[:], tc_t[:])
                    hs_sl = hsum[:, cs : cs + CH_FREE]
                    if t == 0:
                        nc.vector.tensor_copy(hs_sl, hf[:])
                    else:
                        nc.vector.tensor_add(hs_sl, hs_sl, hf[:])
                    if h_new is not None:
                        nc.vector.tensor_copy(
                            h_new[:, 2 + r0 : 2 + r0 + 8, 2:66],
                            hf[:].rearrange("p (r c) -> p r c", r=8),
                        )
                h_pad = h_new

            nc.scalar.mul(hsum[:], hsum[:], 1.0 / T)
            nc.sync.dma_start(out_d[:], hsum[:])

    _nc_cache[0] = nc
    return nc


# ---------------------------------------------------------------------------
# host-side helpers (exact fp32)
# ---------------------------------------------------------------------------


def _conv_np(x, w, pad):
    """x [Ci,H,W], w [Co,Ci,kh,kw] -> [Co,Ho,Wo] fp32, matmul per offset."""
    Co, Ci, kh, kw = w.shape
    Hh, Ww = x.shape[1], x.shape[2]
    xp = np.zeros((Ci, Hh + 2 * pad, Ww + 2 * pad), np.float32)
    xp[:, pad : pad + Hh, pad : pad + Ww] = x
    Ho = Hh + 2 * pad - kh + 1
    Wo = Ww + 2 * pad - kw + 1
    out = np.zeros((Co, Ho * Wo), np.float32)
    for dy in range(kh):
        for dx in range(kw):
            patch = xp[:, dy : dy + Ho, dx : dx + Wo].reshape(Ci, -1)
            out += w[:, :, dy, dx] @ patch
    return out.reshape(Co, Ho, Wo)


def kernel(
    rgb_a,
    confidence_a,
    phi_x_w,
    phi_h_w,
    lstm_w,
    lstm_b,
    conv1_w,
    conv1_b,
    conv2_w,
    conv2_b,
):
    rgb_a = np.asarray(rgb_a, np.float32)
    confidence_a = np.asarray(confidence_a, np.float32)
    lstm_w = np.asarray(lstm_w, np.float32)
    lstm_b = np.asarray(lstm_b, np.float32)

    # --- attention prep (att_h is a constant shift inside softmax -> drop it)
    s = rgb_a * confidence_a
    s = (s - s.min()) / (s.max() - s.min())
    att_x = s.mean(axis=(2, 3)) @ np.asarray(phi_x_w, np.float32)[0]
    e = np.exp(att_x - att_x.max())
    wts = e / e.sum()
    x_t = (s * wts[:, None, None, None]).sum(0) / T  # [3,H,W]

    # --- x-path conv (one-time) and weight layout for the device
    wx = lstm_w[:, :3]
    whh = lstm_w[:, 3:]  # [512,128,5,5]
    gx_full = _conv_np(x_t, wx, 2) + lstm_b[:, None, None]  # [512,64,64]
    gx_in = np.ascontiguousarray(
        gx_full.reshape(4, 128, HW), dtype=np.float32
    )
    # wh[i, og*25*128 + off*128 + o] = whh[og*128+o, i, ky, kx]
    wh_in = np.ascontiguousarray(
        whh.reshape(4, 128, 128, 5, 5).transpose(2, 0, 3, 4, 1).reshape(128, -1)
    ).astype(ml_dtypes.bfloat16)

    nc = build_nc()
    in_map = {"wh": wh_in, "gx": gx_in}
    res = run_bass_kernel_spmd(
        nc,
        [dict(in_map) for _ in range(N_CORES)],
        core_ids=list(range(N_CORES)),
    )
    hmean = res.results[0]["hmean"].reshape(HS, H, W).astype(np.float32)

    # --- CNN tail (host, exact fp32)
    hp = np.full((HS, H + 1, W + 1), -np.inf, np.float32)
    hp[:, :H, :W] = hmean
    views = [
        hp[:, dy : dy + 63 + 1 : 2, dx : dx + 63 + 1 : 2]
        for dy in range(3)
        for dx in range(3)
    ]
    p = np.max(np.stack([v[:, :32, :32] for v in views]), axis=0)

    def sig(v):
        return 1.0 / (1.0 + np.exp(-v))

    y = sig(
        _conv_np(p, np.asarray(conv1_w, np.float32), 3)
        + np.asarray(conv1_b, np.float32)[:, None, None]
    )
    y = sig(
        _conv_np(y, np.asarray(conv2_w, np.float32), 0)
        + np.asarray(conv2_b, np.float32)[:, None, None]
    )
    v = y.sum(axis=(1, 2))
    pred = v / max(np.linalg.norm(v), 1e-12)
    return pred[None].astype(np.float32)
